# revision 1
# baseline (speedup 1.0000x reference)
"""Trainium2 Bass kernel for nn_ComprehensiveGANLoss.

Strategy (8 cores, SPMD, no collectives):
  - local/input features (B=32, S=2048, D=512): shard over S (256 per core).
    Per (b, s)-row: nsq = sum_d x^2 (fused square+reduce on ACT or DVE),
    inv = 1/max(||x||, eps) = sqrt(1/max(nsq, eps^2)).
    Batch sum of normalized rows done on the PE: matmul with a DIAGONAL
    stationary matrix diag(+/- inv) accumulating into PSUM, so PSUM ends up
    holding  B * (fake_mean - real_mean)[s, d]  for the core's s-shard.
    Square+reduce of PSUM -> per-s partial sums -> host.
  - phrase features (B, P=128, D): shard over P (16 per core), same trick with
    a [128, 16] selector (rows are (b, p) pairs).
  - global features (B, D): replicated, computed on every core, host uses
    core 0 (K=32, M=1 matmul with +/- inv as stationary).
  - musical-perceptual term on tokens: shard edges over S with a 1-column
    halo; all indicator math on DVE; exact integer counts in f32.
  - adversarial softplus: ACT Softplus(scale=-1) with fused accumulation.
  Host combines a few KB of partial sums in float64.
"""

import os
import numpy as np

import concourse.bass as bass
import concourse.tile as tile
from concourse import bacc, mybir

B, S, P, D = 32, 2048, 128, 512
NCORES = 8
SL = S // NCORES   # 256 sequence positions per core
PL = P // NCORES   # 16 phrase positions per core

F32 = mybir.dt.float32
F32R = mybir.dt.float32r
AF = mybir.ActivationFunctionType
ALU = mybir.AluOpType

# how many of every 8 row-norm square+reduce ops go to the ACT engine
# (the rest go to DVE via tensor_tensor_reduce)
ACT_OF_8 = int(os.environ.get("K_ACT_OF_8", "4"))
EPS2 = 1e-24  # eps^2 for max(||x||, 1e-12)


def _build(nc):
    """Emit the SPMD device program. Returns (input_names, output_names)."""
    dram = {}

    def din(name, shape, dtype=F32):
        dram[name] = nc.dram_tensor(name, list(shape), dtype, kind="ExternalInput").ap()
        return dram[name]

    def dout(name, shape, dtype=F32):
        dram[name] = nc.dram_tensor(name, list(shape), dtype, kind="ExternalOutput").ap()
        return dram[name]

    # big tensors arrive pre-transposed: (bgroup, sgroup, b16, s8, d)
    BT = (2, 32, 16, 8, D)
    rl = din("rl", BT, F32R); fl = din("fl", BT, F32R)
    ri = din("ri", BT, F32R); fi = din("fi", BT, F32R)
    rp = din("rp", (B, PL, D), F32R); fp = din("fp", (B, PL, D), F32R)
    rg = din("rg", (B, D), F32R);     fg = din("fg", (B, D), F32R)
    ll = din("ll", (B, SL));    pl = din("pl", (B, PL))
    gl = din("gl", (B, 1))
    tok = din("tok", (B, SL + 1))          # tokens as f32, with +1 halo col
    msp = din("msp", (4, 128, 32))         # +selector masks (4 quad variants)
    msn = din("msn", (4, 128, 32))         # -selector masks
    pmp = din("pmp", (128, PL))            # phrase selector mask +
    pmn = din("pmn", (128, PL))            # phrase selector mask -
    onp = din("onp", (B, 1))               # +ones column
    onn = din("onn", (B, 1))               # -ones column

    o_ds = dout("o_ds", (32, 20))          # diff^2 partial sums (per-partition)
    o_mus = dout("o_mus", (B, 4))          # rhythm / harsh6 / harsh11 / melody
    o_adv = dout("o_adv", (B, 3))          # softplus sums: local / phrase / global

    with tile.TileContext(nc) as tc:
        with (
            tc.tile_pool(name="const", bufs=1) as cp,
            tc.tile_pool(name="x", bufs=4) as xp,  # 16KB/part each
            tc.tile_pool(name="norm", bufs=4) as npo,
            tc.tile_pool(name="diag", bufs=4) as dgp,
            tc.tile_pool(name="junk", bufs=2) as jup,
            tc.tile_pool(name="small", bufs=1) as smp,
            tc.tile_pool(name="mus", bufs=1) as mup,
            tc.tile_pool(name="psum", bufs=1, space="PSUM") as psp,
        ):
            # ---- constants into SBUF
            msp_t = cp.tile([128, 4, 32], F32, tag="msp")
            msn_t = cp.tile([128, 4, 32], F32, tag="msn")
            pmp_t = cp.tile([128, PL], F32, tag="pmp")
            pmn_t = cp.tile([128, PL], F32, tag="pmn")
            onp_t = cp.tile([B, 1], F32, tag="onp")
            onn_t = cp.tile([B, 1], F32, tag="onn")
            nc.sync.dma_start(msp_t[:], msp.rearrange("g p m -> p g m"))
            nc.sync.dma_start(msn_t[:], msn.rearrange("g p m -> p g m"))
            nc.sync.dma_start(pmp_t[:], pmp[:])
            nc.sync.dma_start(pmn_t[:], pmn[:])
            nc.sync.dma_start(onp_t[:], onp[:])
            nc.sync.dma_start(onn_t[:], onn[:])

            # ---- output staging tiles
            PARTS = set(os.environ.get(
                "K_PARTS", "pairs,phrase,glob,mus,adv").split(","))
            ods_t = smp.tile([32, 20], F32, tag="ods")
            nc.vector.memset(ods_t[:], 0.0)
            omus_t = mup.tile([B, 4], F32, tag="omus")
            oadv_t = mup.tile([B, 3], F32, tag="oadv")
            nc.vector.memset(omus_t[:], 0.0)
            nc.vector.memset(oadv_t[:], 0.0)

            # ================= big pairs: local & input =================
            def sq_accum(k, xs, nsq_col):
                """nsq_col[128,1] = sum_d xs^2, on ACT or DVE."""
                xf = xs.bitcast(F32)
                if (k % 8) < ACT_OF_8:
                    junk = jup.tile([128, D], F32, tag="junk")
                    nc.scalar.activation(junk[:], xf, AF.Square,
                                         accum_out=nsq_col)
                else:
                    junk = jup.tile([128, D], F32, tag="junk")
                    nc.vector.scalar_tensor_tensor(
                        out=junk[:], in0=xf, scalar=1.0, in1=xf,
                        op0=ALU.mult, op1=ALU.mult, accum_out=nsq_col)

            def inv_norms(nsq, g):
                """inv[128,G] = sqrt(1/max(nsq, EPS2))"""
                G = nsq.shape()[1] if callable(getattr(nsq, "shape", None)) else None
                clip = npo.tile([128, g], F32, tag="clip")
                rec = npo.tile([128, g], F32, tag="rec")
                inv = npo.tile([128, g], F32, tag="inv")
                nc.vector.tensor_scalar(out=clip[:], in0=nsq, scalar1=EPS2,
                                        scalar2=None, op0=ALU.max)
                nc.vector.reciprocal(rec[:], clip[:])
                nc.scalar.activation(inv[:], rec[:], AF.Sqrt)
                return inv

            # tiles hold rows (16 b x 8 s); one 2 MiB DMA covers 8 s-groups.
            # matmul: psum[8g:8g+8, :] += sel[128,8].T @ x[128,512], where
            # sel = (+/-mask) * inv_norm; 4 accumulating MMs per s-group
            # slice (2 b-groups x fake/real).
            kglob = 0
            SGD = 8                         # s-groups per DMA
            pair_list = [(fl, rl, (0, 1)), (fi, ri, (2, 3))]
            if "pairs" not in PARTS:
                pair_list = pair_list[:1] if "pair0" in PARTS else []
            for pair_i, (fake_src, real_src, cols) in enumerate(pair_list):
                # one [32, D] psum tile (own bank) per quad of 4 s-groups
                ps = [psp.tile([32, D], F32, tag=f"q{i}", name=f"psq{i}")
                      for i in range(8)]
                mm_cnt = [0] * 8
                for src, mask in ((fake_src, msp_t), (real_src, msn_t)):
                    for bg in range(2):          # b-groups of 16
                        for q in range(32 // SGD):         # 2 MiB chunks
                            xt = xp.tile([128, SGD, D], F32R, tag="x")
                            nc.sync.dma_start(
                                xt[:],
                                src[bg, q * SGD:(q + 1) * SGD].rearrange(
                                    "g b s d -> (b s) g d"))
                            nsq = npo.tile([128, SGD], F32, tag="nsq")
                            for k in range(SGD):
                                sq_accum(kglob + k, xt[:, k, :],
                                         nsq[:, k:k + 1])
                            kglob += SGD
                            inv = inv_norms(nsq[:], SGD)
                            for k in range(SGD):
                                gi = q * SGD + k          # global s-group
                                quad, gq = gi // 4, gi % 4
                                sel = dgp.tile([128, 32], F32R, tag="sel")
                                nc.gpsimd.tensor_scalar_mul(
                                    sel[:], mask[:, gq, :], inv[:, k:k + 1])
                                cnt = mm_cnt[quad]
                                nc.tensor.matmul(
                                    ps[quad][:], sel[:], xt[:, k, :],
                                    start=(cnt == 0), stop=(cnt == 15))
                                mm_cnt[quad] = cnt + 1
                # drain the 8 quad banks -> per-s partial diff^2 sums
                for j in range(8):
                    col = pair_i * 8 + j
                    junk = jup.tile([32, D], F32, tag="junk32",
                                    name="junk32")
                    nc.scalar.activation(junk[:], ps[j][:], AF.Square,
                                         accum_out=ods_t[:, col:col + 1])

            # ================= phrase features =================
            phrase_list = ((fp, pmp_t), (rp, pmn_t)) if "phrase" in PARTS else ()
            if phrase_list:
                ps_p = psp.tile([PL, D], F32, tag="q0", name="ps_p")
            cnt = 0
            for src, mask in phrase_list:
                pt = xp.tile([128, 4, D], F32R, tag="x")
                nc.sync.dma_start(
                    pt[:], src.rearrange("(t e) p d -> (e p) t d", t=4))
                nsq = npo.tile([128, 4], F32, tag="nsq")
                for t in range(4):
                    junk = jup.tile([128, D], F32, tag="junk")
                    nc.scalar.activation(junk[:], pt[:, t, :].bitcast(F32),
                                         AF.Square, accum_out=nsq[:, t:t + 1])
                inv = inv_norms(nsq[:], 4)
                for t in range(4):
                    selp = dgp.tile([128, PL], F32R, tag="selp")
                    nc.gpsimd.tensor_scalar_mul(selp[:], mask[:], inv[:, t:t + 1])
                    nc.tensor.matmul(
                        ps_p[:], selp[:], pt[:, t, :],
                        start=(cnt == 0), stop=(cnt == 7))
                    cnt += 1
            if phrase_list:
                junk = jup.tile([128, D], F32, tag="junk")
                nc.scalar.activation(junk[:PL, :], ps_p[:], AF.Square,
                                     accum_out=ods_t[:PL, 16:17])

            # ================= global features =================
            glob_list = (((fg, onp_t), (rg, onn_t)) if "glob" in PARTS else ())
            if glob_list:
                ps_g = psp.tile([1, D], F32, tag="q1", name="ps_g")
            for gi, (src, ones) in enumerate(glob_list):
                gt = xp.tile([B, D], F32R, tag="gx")
                nc.sync.dma_start(gt[:], src[:])
                nsq = npo.tile([B, 1], F32, tag="gnsq")
                junk = jup.tile([128, D], F32, tag="junk")
                nc.scalar.activation(junk[:B, :], gt[:].bitcast(F32),
                                     AF.Square, accum_out=nsq[:])
                clip = npo.tile([B, 1], F32, tag="gclip")
                rec = npo.tile([B, 1], F32, tag="grec")
                inv = npo.tile([B, 1], F32, tag="ginv")
                nc.vector.tensor_scalar(out=clip[:], in0=nsq[:], scalar1=EPS2,
                                        scalar2=None, op0=ALU.max)
                nc.vector.reciprocal(rec[:], clip[:])
                nc.scalar.activation(inv[:], rec[:], AF.Sqrt)
                sel = npo.tile([B, 1], F32R, tag="gsel")
                nc.vector.tensor_scalar_mul(sel[:], ones[:], inv[:])
                nc.tensor.matmul(ps_g[:], sel[:], gt[:],
                                 start=(gi == 0), stop=(gi == 1))
            if glob_list:
                junk = jup.tile([128, D], F32, tag="junk")
                nc.scalar.activation(junk[:1, :], ps_g[:], AF.Square,
                                     accum_out=ods_t[:1, 17:18])

            # ================= musical perceptual =================
            if "mus" in PARTS:
                MUS_N = int(os.environ.get("K_MUS_N", "99"))
                tf = mup.tile([B, SL + 1], F32, tag="tf")
                nc.sync.dma_start(tf[:], tok[:])
                W = SL + 1

                def mtile(tag, w=W):
                    return mup.tile([B, w], F32, tag=tag, name=tag)

                a_t = mtile("m_a"); b_t = mtile("m_b"); ts_t = mtile("m_ts")
                nc.vector.tensor_scalar(out=a_t[:], in0=tf[:], scalar1=256.0,
                                        scalar2=None, op0=ALU.is_ge)
                nc.vector.tensor_scalar(out=b_t[:], in0=tf[:], scalar1=768.0,
                                        scalar2=None, op0=ALU.is_ge)
                nc.vector.tensor_sub(ts_t[:], a_t[:], b_t[:])
                junk_m = mtile("m_j1", SL)
                nc.vector.tensor_tensor(out=junk_m[:], in0=ts_t[:, 1:W],
                                        in1=ts_t[:, 0:SL], op=ALU.not_equal)
                nc.vector.tensor_reduce(omus_t[:, 0:1], junk_m[:],
                                        axis=mybir.AxisListType.X, op=ALU.add)

                c_t = mtile("m_c"); p_t = mtile("m_p"); pc_t = mtile("m_pc")
                nc.vector.tensor_scalar(out=c_t[:], in0=tf[:], scalar1=128.0,
                                        scalar2=None, op0=ALU.is_lt)
                nc.vector.tensor_mul(p_t[:], tf[:], c_t[:])
    # pc = p mod 12 via conditional subtraction (exact: p in [0,127])
                cur = p_t
                for ti, tval in enumerate((96.0, 48.0, 24.0, 12.0)):
                    ind_t = mtile(f"m_ind{ti}")
                    nc.vector.tensor_scalar(out=ind_t[:], in0=cur[:],
                                            scalar1=tval, scalar2=None,
                                            op0=ALU.is_ge)
                    nxt = pc_t if ti == 3 else mtile(f"m_mod{ti}")
                    nc.vector.scalar_tensor_tensor(
                        out=nxt[:], in0=ind_t[:], scalar=-tval, in1=cur[:],
                        op0=ALU.mult, op1=ALU.add)
                    cur = nxt

                # |d| == 6 or 11  <=>  d^2 == 36 or 121 (exact small ints)
                d_t = mtile("m_d", SL); iv_t = mtile("m_iv", SL)
                nc.vector.tensor_sub(d_t[:], pc_t[:, 0:SL], pc_t[:, 1:W])
                nc.vector.tensor_mul(iv_t[:], d_t[:], d_t[:])
                junk_m2 = mtile("m_j2", SL)
                nc.vector.tensor_scalar(out=junk_m2[:], in0=iv_t[:], scalar1=36.0,
                                        scalar2=None, op0=ALU.is_equal,
                                        op1=ALU.add, accum_out=omus_t[:, 1:2])
                junk_m3 = mtile("m_j3", SL)
                nc.vector.tensor_scalar(out=junk_m3[:], in0=iv_t[:], scalar1=121.0,
                                        scalar2=None, op0=ALU.is_equal,
                                        op1=ALU.add, accum_out=omus_t[:, 2:3])

                # |pdiff| > 12  <=>  pdiff^2 > 144 (exact: |pdiff| <= 127)
                pd_t = mtile("m_pd", SL); av_t = mtile("m_av", SL)
                nc.vector.tensor_sub(pd_t[:], p_t[:, 0:SL], p_t[:, 1:W])
                nc.vector.tensor_mul(av_t[:], pd_t[:], pd_t[:])
                junk_m4 = mtile("m_j4", SL)
                nc.vector.tensor_scalar(out=junk_m4[:], in0=av_t[:], scalar1=144.0,
                                        scalar2=None, op0=ALU.is_gt,
                                        op1=ALU.add, accum_out=omus_t[:, 3:4])

            # ================= adversarial =================
            if "adv" in PARTS:
                ll_t = mup.tile([B, SL], F32, tag="ll")
                pl_t = mup.tile([B, PL], F32, tag="pl")
                gl_t = mup.tile([B, 1], F32, tag="gl")
                nc.sync.dma_start(ll_t[:], ll[:])
                nc.sync.dma_start(pl_t[:], pl[:])
                nc.sync.dma_start(gl_t[:], gl[:])
    # softplus(-x) = ln(1 + exp(-x)); Exp and Ln share one ACT table set
                for col, (src_t, w) in enumerate(((ll_t, SL), (pl_t, PL),
                                                  (gl_t, 1))):
                    e_t = mup.tile([B, w], F32, tag=f"a_e{col}", name=f"a_e{col}")
                    nc.scalar.activation(e_t[:], src_t[:], AF.Exp, scale=-1.0)
                    junk_a = mup.tile([B, w], F32, tag=f"a_j{col}",
                                      name=f"a_j{col}")
                    nc.scalar.activation(junk_a[:], e_t[:], AF.Ln, bias=1.0,
                                         accum_out=oadv_t[:, col:col + 1])

            # ================= outputs =================
            nc.sync.dma_start(o_ds[:], ods_t[:])
            nc.sync.dma_start(o_mus[:], omus_t[:])
            nc.sync.dma_start(o_adv[:], oadv_t[:])

    in_names = ["rl", "fl", "ri", "fi", "rp", "fp", "rg", "fg",
                "ll", "pl", "gl", "tok", "idp", "idn", "pmp", "pmn",
                "onp", "onn"]
    out_names = ["o_ds", "o_mus", "o_adv"]
    return in_names, out_names


_CACHE = {}


def _get_nc():
    if "nc" not in _CACHE:
        nc = bacc.Bacc("TRN2", target_bir_lowering=False, debug=False,
                       enable_asserts=False, num_devices=NCORES)
        _build(nc)
        nc.compile()
        _CACHE["nc"] = nc
    return _CACHE["nc"]


def _shard_inputs(inputs):
    """Build the 8 per-core input maps from the full input dict."""
    f32 = np.float32
    C = np.ascontiguousarray

    msp = np.zeros((4, 128, 32), dtype=f32)
    for g in range(4):
        for r in range(128):
            msp[g, r, 8 * g + r % 8] = 1.0
    msn = -msp
    pmp = np.zeros((128, PL), dtype=f32)
    for r in range(128):
        pmp[r, r % PL] = 1.0
    pmn = -pmp
    onp = np.ones((B, 1), dtype=f32)
    onn = -onp

    tokens = inputs["tokens"]
    tok_f = tokens.astype(f32)

    in_maps = []
    for c in range(NCORES):
        s0, s1 = c * SL, (c + 1) * SL
        p0, p1 = c * PL, (c + 1) * PL
        if s1 < S:
            tok_c = tok_f[:, s0:s1 + 1]
        else:
            tok_c = np.concatenate([tok_f[:, s0:s1], tok_f[:, s1 - 1:s1]],
                                   axis=1)
        def bigT(x):
            # (B, SL, D) -> (bg, sgroup, b16, s8, D) contiguous
            y = x[:, s0:s1, :].reshape(2, 16, 32, 8, x.shape[-1])
            return C(y.transpose(0, 2, 1, 3, 4))

        in_maps.append({
            "rl": bigT(inputs["real_local"]),
            "fl": bigT(inputs["fake_local"]),
            "ri": bigT(inputs["real_input"]),
            "fi": bigT(inputs["fake_input"]),
            "rp": C(inputs["real_phrase"][:, p0:p1, :]),
            "fp": C(inputs["fake_phrase"][:, p0:p1, :]),
            "rg": C(inputs["real_global"]),
            "fg": C(inputs["fake_global"]),
            "ll": C(inputs["local_logits"][:, s0:s1]),
            "pl": C(inputs["phrase_logits"][:, p0:p1]),
            "gl": C(inputs["global_logits"]),
            "tok": C(tok_c),
            "msp": msp, "msn": msn, "pmp": pmp, "pmn": pmn,
            "onp": onp, "onn": onn,
        })
    return in_maps


def _combine(results):
    """Combine per-core partial sums (float64) into the final scalar."""
    ds = [r["o_ds"].astype(np.float64) for r in results]
    mus = [r["o_mus"].astype(np.float64) for r in results]
    adv = [r["o_adv"].astype(np.float64) for r in results]

    ss_l = sum(d[:, 0:8].sum() for d in ds)
    ss_i = sum(d[:, 8:16].sum() for d in ds)
    ss_p = sum(d[:, 16].sum() for d in ds)
    ss_g = ds[0][:, 17].sum()

    m_l = ss_l / (B * B * S * D)
    m_i = ss_i / (B * B * S * D)
    m_p = ss_p / (B * B * P * D)
    m_g = ss_g / (B * B * D)
    fm = (0.4 * m_l + 0.4 * m_p + 0.2 * m_g + 0.1 * m_i) / 4.0

    rhythm = sum(m[:, 0].sum() for m in mus) / (B * (S - 1))
    harmony = sum(m[:, 1].sum() + m[:, 2].sum() for m in mus) / (B * S)
    melody = sum(m[:, 3].sum() for m in mus) / (B * (S - 1))
    musical = rhythm + harmony + melody

    a_l = sum(a[:, 0].sum() for a in adv) / (B * S)
    a_p = sum(a[:, 1].sum() for a in adv) / (B * P)
    a_g = adv[0][:, 2].sum() / B
    advt = 0.4 * a_l + 0.4 * a_p + 0.2 * a_g

    return np.float32(fm + musical + advt)


def _numpy_core(im):
    """Pure-numpy model of one core's device program (for debugging)."""
    out = {}
    ds = np.zeros((32, 20), np.float64)

    def normed_diff(fake, real, nrows):
        # rows laid out as in the device program
        f = fake.reshape(-1, D).astype(np.float64)
        r = real.reshape(-1, D).astype(np.float64)
        fi = 1.0 / np.maximum(np.sqrt((f * f).sum(-1)), 1e-12)
        ri = 1.0 / np.maximum(np.sqrt((r * r).sum(-1)), 1e-12)
        return f * fi[:, None], r * ri[:, None]

    for pair, (fk, rk) in ((0, ("fl", "rl")), (1, ("fi", "ri"))):
        # im[fk] is (bg, sgroup, b16, s8, D); reassemble to (B, SL, D)
        def unT(y):
            return y.transpose(0, 2, 1, 3, 4).reshape(B, SL, y.shape[-1])
        f = unT(im[fk]).astype(np.float64)
        r = unT(im[rk]).astype(np.float64)
        fn = f / np.maximum(np.linalg.norm(f, axis=-1, keepdims=True), 1e-12)
        rn = r / np.maximum(np.linalg.norm(r, axis=-1, keepdims=True), 1e-12)
        acc = fn.sum(0) - rn.sum(0)          # [SL, D]
        for j in range(8):
            blk = acc[j * 32:(j + 1) * 32]   # [32, D]
            ds[:, pair * 8 + j] += (blk * blk).sum(-1)
    f, r = im["fp"].astype(np.float64), im["rp"].astype(np.float64)
    fn = f / np.maximum(np.linalg.norm(f, axis=-1, keepdims=True), 1e-12)
    rn = r / np.maximum(np.linalg.norm(r, axis=-1, keepdims=True), 1e-12)
    acc = fn.sum(0) - rn.sum(0)              # [PL, D]
    ds[:PL, 16] = (acc * acc).sum(-1)
    f, r = im["fg"].astype(np.float64), im["rg"].astype(np.float64)
    fn = f / np.maximum(np.linalg.norm(f, axis=-1, keepdims=True), 1e-12)
    rn = r / np.maximum(np.linalg.norm(r, axis=-1, keepdims=True), 1e-12)
    acc = fn.sum(0) - rn.sum(0)              # [D]
    ds[0, 17] = (acc * acc).sum()
    out["o_ds"] = ds

    t = im["tok"].astype(np.float64)
    tsh = ((t >= 256) & (t < 768)).astype(np.float64)
    mus = np.zeros((B, 4), np.float64)
    mus[:, 0] = np.abs(np.diff(tsh, axis=1)).sum(1)
    p = t * (t < 128)
    pc = np.mod(p, 12.0)
    iv = np.abs(pc[:, :-1] - pc[:, 1:])
    mus[:, 1] = (iv == 6).sum(1)
    mus[:, 2] = (iv == 11).sum(1)
    pd = np.abs(p[:, :-1] - p[:, 1:])
    mus[:, 3] = (pd > 12).sum(1)
    out["o_mus"] = mus

    sp = lambda x: np.log1p(np.exp(-np.abs(x))) + np.maximum(-x, 0.0)
    adv = np.zeros((B, 3), np.float64)
    adv[:, 0] = sp(im["ll"].astype(np.float64)).sum(1)
    adv[:, 1] = sp(im["pl"].astype(np.float64)).sum(1)
    adv[:, 2] = sp(im["gl"].astype(np.float64)).sum(1)
    out["o_adv"] = adv
    return out


def _run(inputs, backend="hw", trace=False):
    """Returns (scalar_result, exec_time_ns_or_None, raw_results)."""
    in_maps = _shard_inputs(inputs)
    if backend == "numpy":
        results = [_numpy_core(im) for im in in_maps]
        return _combine(results), None, results
    nc = _get_nc()
    if backend == "sim":
        from concourse.bass_interp import CoreSim
        results = []
        for im in in_maps:
            sim = CoreSim(nc, trace=False)
            for k, v in im.items():
                sim.tensor(k)[:] = v
            sim.simulate()
            results.append({k: np.array(sim.tensor(k))
                            for k in ("o_ds", "o_mus", "o_adv")})
        return _combine(results), None, results
    from concourse.bass_utils import run_bass_kernel_spmd
    br = run_bass_kernel_spmd(nc, in_maps, list(range(NCORES)), trace=trace)
    return _combine(br.results), br.exec_time_ns, br.results


def kernel(**inputs) -> np.ndarray:
    result, _, _ = _run(inputs, backend="hw")
    return result



# revision 9
# speedup vs baseline: 1.2391x; 1.2391x over previous
"""Trainium2 Bass kernel for nn_ComprehensiveGANLoss.

Strategy (8 cores, SPMD, no collectives):
  - local/input features (B=32, S=2048, D=512): shard over S (256 per core).
    Per (b, s)-row: nsq = sum_d x^2 (fused square+reduce on ACT or DVE),
    inv = 1/max(||x||, eps) = sqrt(1/max(nsq, eps^2)).
    Batch sum of normalized rows done on the PE: matmul with a DIAGONAL
    stationary matrix diag(+/- inv) accumulating into PSUM, so PSUM ends up
    holding  B * (fake_mean - real_mean)[s, d]  for the core's s-shard.
    Square+reduce of PSUM -> per-s partial sums -> host.
  - phrase features (B, P=128, D): shard over P (16 per core), same trick with
    a [128, 16] selector (rows are (b, p) pairs).
  - global features (B, D): replicated, computed on every core, host uses
    core 0 (K=32, M=1 matmul with +/- inv as stationary).
  - musical-perceptual term on tokens: shard edges over S with a 1-column
    halo; all indicator math on DVE; exact integer counts in f32.
  - adversarial softplus: ACT Softplus(scale=-1) with fused accumulation.
  Host combines a few KB of partial sums in float64.
"""

import os
import numpy as np

import concourse.bass as bass
import concourse.tile as tile
from concourse import bacc, mybir

B, S, P, D = 32, 2048, 128, 512
NCORES = 8
SL = S // NCORES   # 256 sequence positions per core
PL = P // NCORES   # 16 phrase positions per core

F32 = mybir.dt.float32
F32R = mybir.dt.float32r
AF = mybir.ActivationFunctionType
ALU = mybir.AluOpType

# how many of every 8 row-norm square+reduce ops go to the ACT engine
# (the rest go to DVE via tensor_tensor_reduce)
ACT_OF_8 = int(os.environ.get("K_ACT_OF_8", "4"))
EPS2 = 1e-24  # eps^2 for max(||x||, 1e-12)


def _build(nc):
    """Emit the SPMD device program. Returns (input_names, output_names)."""
    dram = {}

    def din(name, shape, dtype=F32):
        dram[name] = nc.dram_tensor(name, list(shape), dtype, kind="ExternalInput").ap()
        return dram[name]

    def dout(name, shape, dtype=F32):
        dram[name] = nc.dram_tensor(name, list(shape), dtype, kind="ExternalOutput").ap()
        return dram[name]

    # big tensors arrive pre-transposed to [128, 32768]:
    #   row  = (b16, s8) pair (b within b-group, s within s-group)
    #   cols = (bgroup, sgroup, d) -> every [128, 512] slice is one
    #          (bg, sg) group, and any run of slices is DRAM-contiguous
    #          per partition (16 KiB lines for an 8-slice tile).
    BT = (128, 2 * 32 * D)
    rl = din("rl", BT, F32R); fl = din("fl", BT, F32R)
    ri = din("ri", BT, F32R); fi = din("fi", BT, F32R)
    rp = din("rp", (128, 4 * D), F32R); fp = din("fp", (128, 4 * D), F32R)
    rg = din("rg", (B, D), F32R);     fg = din("fg", (B, D), F32R)
    ll = din("ll", (B, SL));    pl = din("pl", (B, PL))
    gl = din("gl", (B, 1))
    tok = din("tok", (B, SL + 1))          # tokens as f32, with +1 halo col
    msp = din("msp", (4, 128, 32))         # +selector masks (4 quad variants)
    msn = din("msn", (4, 128, 32))         # -selector masks
    pmp = din("pmp", (128, PL))            # phrase selector mask +
    pmn = din("pmn", (128, PL))            # phrase selector mask -
    onp = din("onp", (B, 1))               # +ones column
    onn = din("onn", (B, 1))               # -ones column

    o_ds = dout("o_ds", (32, 20))          # diff^2 partial sums (per-partition)
    o_mus = dout("o_mus", (B, 4))          # rhythm / harsh6 / harsh11 / melody
    o_adv = dout("o_adv", (B, 3))          # softplus sums: local / phrase / global

    with tile.TileContext(nc) as tc:
        with (
            tc.tile_pool(name="const", bufs=1) as cp,
            tc.tile_pool(name="x", bufs=4) as xp,  # 16KB/part each
            tc.tile_pool(name="norm", bufs=4) as npo,
            tc.tile_pool(name="diag", bufs=4) as dgp,
            tc.tile_pool(name="junk", bufs=2) as jup,
            tc.tile_pool(name="small", bufs=1) as smp,
            tc.tile_pool(name="mus", bufs=1) as mup,
            tc.tile_pool(name="psum", bufs=1, space="PSUM") as psp,
        ):
            # ---- constants into SBUF
            msp_t = cp.tile([128, 4, 32], F32, tag="msp")
            msn_t = cp.tile([128, 4, 32], F32, tag="msn")
            pmp_t = cp.tile([128, PL], F32, tag="pmp")
            pmn_t = cp.tile([128, PL], F32, tag="pmn")
            onp_t = cp.tile([B, 1], F32, tag="onp")
            onn_t = cp.tile([B, 1], F32, tag="onn")
            nc.sync.dma_start(msp_t[:], msp.rearrange("g p m -> p g m"))
            nc.sync.dma_start(msn_t[:], msn.rearrange("g p m -> p g m"))
            nc.sync.dma_start(pmp_t[:], pmp[:])
            nc.sync.dma_start(pmn_t[:], pmn[:])
            nc.sync.dma_start(onp_t[:], onp[:])
            nc.sync.dma_start(onn_t[:], onn[:])

            # ---- output staging tiles
            PARTS = set(os.environ.get(
                "K_PARTS", "pairs,phrase,glob,mus,adv").split(","))
            ods_t = smp.tile([32, 20], F32, tag="ods")
            nc.vector.memset(ods_t[:], 0.0)
            omus_t = mup.tile([B, 4], F32, tag="omus")
            oadv_t = mup.tile([B, 3], F32, tag="oadv")
            nc.vector.memset(omus_t[:], 0.0)
            nc.vector.memset(oadv_t[:], 0.0)

            # ================= big pairs: local & input =================
            def sq_accum(k, xs, nsq_col):
                """nsq_col[128,1] = sum_d xs^2, on ACT or DVE."""
                xf = xs.bitcast(F32)
                if (k % 8) < ACT_OF_8:
                    junk = jup.tile([128, D], F32, tag="junk")
                    nc.scalar.activation(junk[:], xf, AF.Square,
                                         accum_out=nsq_col)
                else:
                    junk = jup.tile([128, D], F32, tag="junk")
                    nc.vector.scalar_tensor_tensor(
                        out=junk[:], in0=xf, scalar=1.0, in1=xf,
                        op0=ALU.mult, op1=ALU.mult, accum_out=nsq_col)

            def inv_norms(nsq, g):
                """inv[128,G] = sqrt(1/max(nsq, EPS2))"""
                G = nsq.shape()[1] if callable(getattr(nsq, "shape", None)) else None
                clip = npo.tile([128, g], F32, tag="clip")
                rec = npo.tile([128, g], F32, tag="rec")
                inv = npo.tile([128, g], F32, tag="inv")
                nc.vector.tensor_scalar(out=clip[:], in0=nsq, scalar1=EPS2,
                                        scalar2=None, op0=ALU.max)
                nc.vector.reciprocal(rec[:], clip[:])
                nc.scalar.activation(inv[:], rec[:], AF.Sqrt)
                return inv

            # tiles hold rows (16 b x 8 s); one 2 MiB contiguous DMA covers
            # 8 (bg, sg) group slices. DMAs alternate between the two HWDGE
            # rings (SP via nc.sync, ACT via nc.scalar) so the rings overlap.
            # matmul: psum[8g:8g+8, :] += sel[128,8].T @ x[128,512], where
            # sel = (+/-mask) * inv_norm; 4 accumulating MMs per s-group
            # slice (2 b-groups x fake/real).
            kglob = 0
            SGD = 8                         # (bg, sg) slices per DMA
            dma_engs = {"sync": [nc.sync],
                        "alt": [nc.sync, nc.scalar],
                        "alt3": [nc.sync, nc.scalar, nc.gpsimd]}[
                os.environ.get("K_DMA", "alt")]
            pair_list = [(fl, rl, (0, 1)), (fi, ri, (2, 3))]
            if "pairs" not in PARTS:
                pair_list = pair_list[:1] if "pair0" in PARTS else []
            for pair_i, (fake_src, real_src, cols) in enumerate(pair_list):
                # one [32, D] psum tile (own bank) per quad of 4 s-groups
                ps = [psp.tile([32, D], F32, tag=f"q{i}", name=f"psq{i}")
                      for i in range(8)]
                mm_cnt = [0] * 8
                for src, mask in ((fake_src, msp_t), (real_src, msn_t)):
                    for bg in range(2):          # b-groups of 16
                        for q in range(32 // SGD):         # 2 MiB chunks
                            xt = xp.tile([128, SGD * D], F32R, tag="x")
                            off = (bg * 32 + q * SGD) * D
                            dma_engs[(kglob // SGD) % len(dma_engs)].dma_start(
                                xt[:], src[:, off:off + SGD * D])
                            nsq = npo.tile([128, SGD], F32, tag="nsq")
                            for k in range(SGD):
                                sq_accum(kglob + k, xt[:, k * D:(k + 1) * D],
                                         nsq[:, k:k + 1])
                            kglob += SGD
                            inv = inv_norms(nsq[:], SGD)
                            for k in range(SGD):
                                gi = q * SGD + k          # global s-group
                                quad, gq = gi // 4, gi % 4
                                sel = dgp.tile([128, 32], F32R, tag="sel")
                                nc.gpsimd.tensor_scalar_mul(
                                    sel[:], mask[:, gq, :], inv[:, k:k + 1])
                                cnt = mm_cnt[quad]
                                nc.tensor.matmul(
                                    ps[quad][:], sel[:],
                                    xt[:, k * D:(k + 1) * D],
                                    start=(cnt == 0), stop=(cnt == 15))
                                mm_cnt[quad] = cnt + 1
                # drain the 8 quad banks -> per-s partial diff^2 sums
                for j in range(8):
                    col = pair_i * 8 + j
                    junk = jup.tile([32, D], F32, tag="junk32",
                                    name="junk32")
                    nc.scalar.activation(junk[:], ps[j][:], AF.Square,
                                         accum_out=ods_t[:, col:col + 1])

            # ================= phrase features =================
            phrase_list = ((fp, pmp_t), (rp, pmn_t)) if "phrase" in PARTS else ()
            if phrase_list:
                ps_p = psp.tile([PL, D], F32, tag="q0", name="ps_p")
            cnt = 0
            for src, mask in phrase_list:
                pt = xp.tile([128, 4 * D], F32R, tag="x")
                nc.sync.dma_start(pt[:], src[:])
                nsq = npo.tile([128, 4], F32, tag="nsq")
                for t in range(4):
                    junk = jup.tile([128, D], F32, tag="junk")
                    nc.scalar.activation(junk[:],
                                         pt[:, t * D:(t + 1) * D].bitcast(F32),
                                         AF.Square, accum_out=nsq[:, t:t + 1])
                inv = inv_norms(nsq[:], 4)
                for t in range(4):
                    selp = dgp.tile([128, PL], F32R, tag="selp")
                    nc.gpsimd.tensor_scalar_mul(selp[:], mask[:], inv[:, t:t + 1])
                    nc.tensor.matmul(
                        ps_p[:], selp[:], pt[:, t * D:(t + 1) * D],
                        start=(cnt == 0), stop=(cnt == 7))
                    cnt += 1
            if phrase_list:
                junk = jup.tile([128, D], F32, tag="junk")
                nc.scalar.activation(junk[:PL, :], ps_p[:], AF.Square,
                                     accum_out=ods_t[:PL, 16:17])

            # ================= global features =================
            glob_list = (((fg, onp_t), (rg, onn_t)) if "glob" in PARTS else ())
            if glob_list:
                ps_g = psp.tile([1, D], F32, tag="q1", name="ps_g")
            for gi, (src, ones) in enumerate(glob_list):
                gt = xp.tile([B, D], F32R, tag="gx")
                nc.sync.dma_start(gt[:], src[:])
                nsq = npo.tile([B, 1], F32, tag="gnsq")
                junk = jup.tile([128, D], F32, tag="junk")
                nc.scalar.activation(junk[:B, :], gt[:].bitcast(F32),
                                     AF.Square, accum_out=nsq[:])
                clip = npo.tile([B, 1], F32, tag="gclip")
                rec = npo.tile([B, 1], F32, tag="grec")
                inv = npo.tile([B, 1], F32, tag="ginv")
                nc.vector.tensor_scalar(out=clip[:], in0=nsq[:], scalar1=EPS2,
                                        scalar2=None, op0=ALU.max)
                nc.vector.reciprocal(rec[:], clip[:])
                nc.scalar.activation(inv[:], rec[:], AF.Sqrt)
                sel = npo.tile([B, 1], F32R, tag="gsel")
                nc.vector.tensor_scalar_mul(sel[:], ones[:], inv[:])
                nc.tensor.matmul(ps_g[:], sel[:], gt[:],
                                 start=(gi == 0), stop=(gi == 1))
            if glob_list:
                junk = jup.tile([128, D], F32, tag="junk")
                nc.scalar.activation(junk[:1, :], ps_g[:], AF.Square,
                                     accum_out=ods_t[:1, 17:18])

            # ================= musical perceptual =================
            if "mus" in PARTS:
                MUS_N = int(os.environ.get("K_MUS_N", "99"))
                tf = mup.tile([B, SL + 1], F32, tag="tf")
                nc.sync.dma_start(tf[:], tok[:])
                W = SL + 1

                def mtile(tag, w=W):
                    return mup.tile([B, w], F32, tag=tag, name=tag)

                a_t = mtile("m_a"); b_t = mtile("m_b"); ts_t = mtile("m_ts")
                nc.vector.tensor_scalar(out=a_t[:], in0=tf[:], scalar1=256.0,
                                        scalar2=None, op0=ALU.is_ge)
                nc.vector.tensor_scalar(out=b_t[:], in0=tf[:], scalar1=768.0,
                                        scalar2=None, op0=ALU.is_ge)
                nc.vector.tensor_sub(ts_t[:], a_t[:], b_t[:])
                junk_m = mtile("m_j1", SL)
                nc.vector.tensor_tensor(out=junk_m[:], in0=ts_t[:, 1:W],
                                        in1=ts_t[:, 0:SL], op=ALU.not_equal)
                nc.vector.tensor_reduce(omus_t[:, 0:1], junk_m[:],
                                        axis=mybir.AxisListType.X, op=ALU.add)

                c_t = mtile("m_c"); p_t = mtile("m_p"); pc_t = mtile("m_pc")
                nc.vector.tensor_scalar(out=c_t[:], in0=tf[:], scalar1=128.0,
                                        scalar2=None, op0=ALU.is_lt)
                nc.vector.tensor_mul(p_t[:], tf[:], c_t[:])
    # pc = p mod 12 via conditional subtraction (exact: p in [0,127])
                cur = p_t
                for ti, tval in enumerate((96.0, 48.0, 24.0, 12.0)):
                    ind_t = mtile(f"m_ind{ti}")
                    nc.vector.tensor_scalar(out=ind_t[:], in0=cur[:],
                                            scalar1=tval, scalar2=None,
                                            op0=ALU.is_ge)
                    nxt = pc_t if ti == 3 else mtile(f"m_mod{ti}")
                    nc.vector.scalar_tensor_tensor(
                        out=nxt[:], in0=ind_t[:], scalar=-tval, in1=cur[:],
                        op0=ALU.mult, op1=ALU.add)
                    cur = nxt

                # |d| == 6 or 11  <=>  d^2 == 36 or 121 (exact small ints)
                d_t = mtile("m_d", SL); iv_t = mtile("m_iv", SL)
                nc.vector.tensor_sub(d_t[:], pc_t[:, 0:SL], pc_t[:, 1:W])
                nc.vector.tensor_mul(iv_t[:], d_t[:], d_t[:])
                junk_m2 = mtile("m_j2", SL)
                nc.vector.tensor_scalar(out=junk_m2[:], in0=iv_t[:], scalar1=36.0,
                                        scalar2=None, op0=ALU.is_equal,
                                        op1=ALU.add, accum_out=omus_t[:, 1:2])
                junk_m3 = mtile("m_j3", SL)
                nc.vector.tensor_scalar(out=junk_m3[:], in0=iv_t[:], scalar1=121.0,
                                        scalar2=None, op0=ALU.is_equal,
                                        op1=ALU.add, accum_out=omus_t[:, 2:3])

                # |pdiff| > 12  <=>  pdiff^2 > 144 (exact: |pdiff| <= 127)
                pd_t = mtile("m_pd", SL); av_t = mtile("m_av", SL)
                nc.vector.tensor_sub(pd_t[:], p_t[:, 0:SL], p_t[:, 1:W])
                nc.vector.tensor_mul(av_t[:], pd_t[:], pd_t[:])
                junk_m4 = mtile("m_j4", SL)
                nc.vector.tensor_scalar(out=junk_m4[:], in0=av_t[:], scalar1=144.0,
                                        scalar2=None, op0=ALU.is_gt,
                                        op1=ALU.add, accum_out=omus_t[:, 3:4])

            # ================= adversarial =================
            if "adv" in PARTS:
                ll_t = mup.tile([B, SL], F32, tag="ll")
                pl_t = mup.tile([B, PL], F32, tag="pl")
                gl_t = mup.tile([B, 1], F32, tag="gl")
                nc.sync.dma_start(ll_t[:], ll[:])
                nc.sync.dma_start(pl_t[:], pl[:])
                nc.sync.dma_start(gl_t[:], gl[:])
    # softplus(-x) = ln(1 + exp(-x)); Exp and Ln share one ACT table set
                for col, (src_t, w) in enumerate(((ll_t, SL), (pl_t, PL),
                                                  (gl_t, 1))):
                    e_t = mup.tile([B, w], F32, tag=f"a_e{col}", name=f"a_e{col}")
                    nc.scalar.activation(e_t[:], src_t[:], AF.Exp, scale=-1.0)
                    junk_a = mup.tile([B, w], F32, tag=f"a_j{col}",
                                      name=f"a_j{col}")
                    nc.scalar.activation(junk_a[:], e_t[:], AF.Ln, bias=1.0,
                                         accum_out=oadv_t[:, col:col + 1])

            # ================= outputs =================
            nc.sync.dma_start(o_ds[:], ods_t[:])
            nc.sync.dma_start(o_mus[:], omus_t[:])
            nc.sync.dma_start(o_adv[:], oadv_t[:])

    in_names = ["rl", "fl", "ri", "fi", "rp", "fp", "rg", "fg",
                "ll", "pl", "gl", "tok", "idp", "idn", "pmp", "pmn",
                "onp", "onn"]
    out_names = ["o_ds", "o_mus", "o_adv"]
    return in_names, out_names


_CACHE = {}


def _get_nc():
    if "nc" not in _CACHE:
        nc = bacc.Bacc("TRN2", target_bir_lowering=False, debug=False,
                       enable_asserts=False, num_devices=NCORES)
        _build(nc)
        nc.compile()
        _CACHE["nc"] = nc
    return _CACHE["nc"]


def _shard_inputs(inputs):
    """Build the 8 per-core input maps from the full input dict."""
    f32 = np.float32
    C = np.ascontiguousarray

    msp = np.zeros((4, 128, 32), dtype=f32)
    for g in range(4):
        for r in range(128):
            msp[g, r, 8 * g + r % 8] = 1.0
    msn = -msp
    pmp = np.zeros((128, PL), dtype=f32)
    for r in range(128):
        pmp[r, r % PL] = 1.0
    pmn = -pmp
    onp = np.ones((B, 1), dtype=f32)
    onn = -onp

    tokens = inputs["tokens"]
    tok_f = tokens.astype(f32)

    in_maps = []
    for c in range(NCORES):
        s0, s1 = c * SL, (c + 1) * SL
        p0, p1 = c * PL, (c + 1) * PL
        if s1 < S:
            tok_c = tok_f[:, s0:s1 + 1]
        else:
            tok_c = np.concatenate([tok_f[:, s0:s1], tok_f[:, s1 - 1:s1]],
                                   axis=1)
        def bigT(x):
            # (B, SL, D) -> [128, (bg, sg, d)] contiguous:
            #   row (b16, s8), col ((bg*32 + sg)*D + d)
            y = x[:, s0:s1, :].reshape(2, 16, 32, 8, x.shape[-1])
            return C(y.transpose(1, 3, 0, 2, 4).reshape(128, 2 * 32 * D))

        def phrT(x):
            # (B, PL, D) -> [128, (t, d)]: row (e8, p16), col (t*D + d)
            # where b = t*8 + e
            y = x[:, p0:p1, :].reshape(4, 8, PL, D)
            return C(y.transpose(1, 2, 0, 3).reshape(128, 4 * D))

        in_maps.append({
            "rl": bigT(inputs["real_local"]),
            "fl": bigT(inputs["fake_local"]),
            "ri": bigT(inputs["real_input"]),
            "fi": bigT(inputs["fake_input"]),
            "rp": phrT(inputs["real_phrase"]),
            "fp": phrT(inputs["fake_phrase"]),
            "rg": C(inputs["real_global"]),
            "fg": C(inputs["fake_global"]),
            "ll": C(inputs["local_logits"][:, s0:s1]),
            "pl": C(inputs["phrase_logits"][:, p0:p1]),
            "gl": C(inputs["global_logits"]),
            "tok": C(tok_c),
            "msp": msp, "msn": msn, "pmp": pmp, "pmn": pmn,
            "onp": onp, "onn": onn,
        })
    return in_maps


def _combine(results):
    """Combine per-core partial sums (float64) into the final scalar."""
    ds = [r["o_ds"].astype(np.float64) for r in results]
    mus = [r["o_mus"].astype(np.float64) for r in results]
    adv = [r["o_adv"].astype(np.float64) for r in results]

    ss_l = sum(d[:, 0:8].sum() for d in ds)
    ss_i = sum(d[:, 8:16].sum() for d in ds)
    ss_p = sum(d[:, 16].sum() for d in ds)
    ss_g = ds[0][:, 17].sum()

    m_l = ss_l / (B * B * S * D)
    m_i = ss_i / (B * B * S * D)
    m_p = ss_p / (B * B * P * D)
    m_g = ss_g / (B * B * D)
    fm = (0.4 * m_l + 0.4 * m_p + 0.2 * m_g + 0.1 * m_i) / 4.0

    rhythm = sum(m[:, 0].sum() for m in mus) / (B * (S - 1))
    harmony = sum(m[:, 1].sum() + m[:, 2].sum() for m in mus) / (B * S)
    melody = sum(m[:, 3].sum() for m in mus) / (B * (S - 1))
    musical = rhythm + harmony + melody

    a_l = sum(a[:, 0].sum() for a in adv) / (B * S)
    a_p = sum(a[:, 1].sum() for a in adv) / (B * P)
    a_g = adv[0][:, 2].sum() / B
    advt = 0.4 * a_l + 0.4 * a_p + 0.2 * a_g

    return np.float32(fm + musical + advt)


def _numpy_core(im):
    """Pure-numpy model of one core's device program (for debugging)."""
    out = {}
    ds = np.zeros((32, 20), np.float64)

    def normed_diff(fake, real, nrows):
        # rows laid out as in the device program
        f = fake.reshape(-1, D).astype(np.float64)
        r = real.reshape(-1, D).astype(np.float64)
        fi = 1.0 / np.maximum(np.sqrt((f * f).sum(-1)), 1e-12)
        ri = 1.0 / np.maximum(np.sqrt((r * r).sum(-1)), 1e-12)
        return f * fi[:, None], r * ri[:, None]

    for pair, (fk, rk) in ((0, ("fl", "rl")), (1, ("fi", "ri"))):
        # im[fk] is [128, (bg, sg, d)]; reassemble to (B, SL, D)
        def unT(y):
            z = y.reshape(16, 8, 2, 32, D)          # b, s, bg, sg, d
            return z.transpose(2, 0, 3, 1, 4).reshape(B, SL, D)
        f = unT(im[fk]).astype(np.float64)
        r = unT(im[rk]).astype(np.float64)
        fn = f / np.maximum(np.linalg.norm(f, axis=-1, keepdims=True), 1e-12)
        rn = r / np.maximum(np.linalg.norm(r, axis=-1, keepdims=True), 1e-12)
        acc = fn.sum(0) - rn.sum(0)          # [SL, D]
        for j in range(8):
            blk = acc[j * 32:(j + 1) * 32]   # [32, D]
            ds[:, pair * 8 + j] += (blk * blk).sum(-1)
    def unP(y):
        z = y.reshape(8, PL, 4, D)               # e, p, t, d
        return z.transpose(2, 0, 1, 3).reshape(B, PL, D)
    f = unP(im["fp"]).astype(np.float64)
    r = unP(im["rp"]).astype(np.float64)
    fn = f / np.maximum(np.linalg.norm(f, axis=-1, keepdims=True), 1e-12)
    rn = r / np.maximum(np.linalg.norm(r, axis=-1, keepdims=True), 1e-12)
    acc = fn.sum(0) - rn.sum(0)              # [PL, D]
    ds[:PL, 16] = (acc * acc).sum(-1)
    f, r = im["fg"].astype(np.float64), im["rg"].astype(np.float64)
    fn = f / np.maximum(np.linalg.norm(f, axis=-1, keepdims=True), 1e-12)
    rn = r / np.maximum(np.linalg.norm(r, axis=-1, keepdims=True), 1e-12)
    acc = fn.sum(0) - rn.sum(0)              # [D]
    ds[0, 17] = (acc * acc).sum()
    out["o_ds"] = ds

    t = im["tok"].astype(np.float64)
    tsh = ((t >= 256) & (t < 768)).astype(np.float64)
    mus = np.zeros((B, 4), np.float64)
    mus[:, 0] = np.abs(np.diff(tsh, axis=1)).sum(1)
    p = t * (t < 128)
    pc = np.mod(p, 12.0)
    iv = np.abs(pc[:, :-1] - pc[:, 1:])
    mus[:, 1] = (iv == 6).sum(1)
    mus[:, 2] = (iv == 11).sum(1)
    pd = np.abs(p[:, :-1] - p[:, 1:])
    mus[:, 3] = (pd > 12).sum(1)
    out["o_mus"] = mus

    sp = lambda x: np.log1p(np.exp(-np.abs(x))) + np.maximum(-x, 0.0)
    adv = np.zeros((B, 3), np.float64)
    adv[:, 0] = sp(im["ll"].astype(np.float64)).sum(1)
    adv[:, 1] = sp(im["pl"].astype(np.float64)).sum(1)
    adv[:, 2] = sp(im["gl"].astype(np.float64)).sum(1)
    out["o_adv"] = adv
    return out


def _run(inputs, backend="hw", trace=False):
    """Returns (scalar_result, exec_time_ns_or_None, raw_results)."""
    in_maps = _shard_inputs(inputs)
    if backend == "numpy":
        results = [_numpy_core(im) for im in in_maps]
        return _combine(results), None, results
    nc = _get_nc()
    if backend == "sim":
        from concourse.bass_interp import CoreSim
        results = []
        for im in in_maps:
            sim = CoreSim(nc, trace=False)
            for k, v in im.items():
                sim.tensor(k)[:] = v
            sim.simulate()
            results.append({k: np.array(sim.tensor(k))
                            for k in ("o_ds", "o_mus", "o_adv")})
        return _combine(results), None, results
    from concourse.bass_utils import run_bass_kernel_spmd
    br = run_bass_kernel_spmd(nc, in_maps, list(range(NCORES)), trace=trace)
    return _combine(br.results), br.exec_time_ns, br.results


def kernel(**inputs) -> np.ndarray:
    result, _, _ = _run(inputs, backend="hw")
    return result



# revision 13
# speedup vs baseline: 1.5358x; 1.2395x over previous
"""Trainium2 Bass kernel for nn_ComprehensiveGANLoss.

Strategy (8 cores, SPMD, no collectives):
  - local/input features (B=32, S=2048, D=512): shard over S (256 per core).
    Per (b, s)-row: nsq = sum_d x^2 (fused square+reduce on ACT or DVE),
    inv = 1/max(||x||, eps) = sqrt(1/max(nsq, eps^2)).
    Batch sum of normalized rows done on the PE: matmul with a DIAGONAL
    stationary matrix diag(+/- inv) accumulating into PSUM, so PSUM ends up
    holding  B * (fake_mean - real_mean)[s, d]  for the core's s-shard.
    Square+reduce of PSUM -> per-s partial sums -> host.
  - phrase features (B, P=128, D): shard over P (16 per core), same trick with
    a [128, 16] selector (rows are (b, p) pairs).
  - global features (B, D): replicated, computed on every core, host uses
    core 0 (K=32, M=1 matmul with +/- inv as stationary).
  - musical-perceptual term on tokens: shard edges over S with a 1-column
    halo; all indicator math on DVE; exact integer counts in f32.
  - adversarial softplus: ACT Softplus(scale=-1) with fused accumulation.
  Host combines a few KB of partial sums in float64.
"""

import os
import numpy as np

import concourse.bass as bass
import concourse.tile as tile
from concourse import bacc, mybir

B, S, P, D = 32, 2048, 128, 512
NCORES = 8
SL = S // NCORES   # 256 sequence positions per core
PL = P // NCORES   # 16 phrase positions per core

F32 = mybir.dt.float32
F32R = mybir.dt.float32r
AF = mybir.ActivationFunctionType
ALU = mybir.AluOpType

# how many of every 8 row-norm square+reduce ops go to the ACT engine
# (the rest go to DVE via tensor_tensor_reduce)
ACT_OF_8 = int(os.environ.get("K_ACT_OF_8", "4"))
# square+reduce mode: "slice" = 8 per-slice ops (ACT/DVE split per
# ACT_OF_8); "seg" = per tile, one DVE multiply + one segmented
# tensor_reduce; K_ACT_TILES_OF_8 of every 8 tiles instead use the
# ACT slice path to balance engines.
SQMODE = os.environ.get("K_SQ", "seg")
ACT_TILES_OF_8 = int(os.environ.get("K_ACT_TILES_OF_8", "0"))
EPS2 = 1e-24  # eps^2 for max(||x||, 1e-12)


def _build(nc):
    """Emit the SPMD device program. Returns (input_names, output_names)."""
    dram = {}

    def din(name, shape, dtype=F32):
        dram[name] = nc.dram_tensor(name, list(shape), dtype, kind="ExternalInput").ap()
        return dram[name]

    def dout(name, shape, dtype=F32):
        dram[name] = nc.dram_tensor(name, list(shape), dtype, kind="ExternalOutput").ap()
        return dram[name]

    # big tensors arrive pre-transposed to [128, 32768]:
    #   row  = (b16, s8) pair (b within b-group, s within s-group)
    #   cols = (bgroup, sgroup, d) -> every [128, 512] slice is one
    #          (bg, sg) group, and any run of slices is DRAM-contiguous
    #          per partition (16 KiB lines for an 8-slice tile).
    BT = (128, 2 * 32 * D)
    rl = din("rl", BT, F32R); fl = din("fl", BT, F32R)
    ri = din("ri", BT, F32R); fi = din("fi", BT, F32R)
    rp = din("rp", (128, 4 * D), F32R); fp = din("fp", (128, 4 * D), F32R)
    rg = din("rg", (B, D), F32R);     fg = din("fg", (B, D), F32R)
    ll = din("ll", (B, SL));    pl = din("pl", (B, PL))
    gl = din("gl", (B, 1))
    tok = din("tok", (B, SL + 1))          # tokens as f32, with +1 halo col
    msp = din("msp", (4, 128, 32))         # +selector masks (4 quad variants)
    msn = din("msn", (4, 128, 32))         # -selector masks
    pmp = din("pmp", (128, PL))            # phrase selector mask +
    pmn = din("pmn", (128, PL))            # phrase selector mask -
    onp = din("onp", (B, 1))               # +ones column
    onn = din("onn", (B, 1))               # -ones column

    o_ds = dout("o_ds", (32, 20))          # diff^2 partial sums (per-partition)
    o_mus = dout("o_mus", (B, 4))          # rhythm / harsh6 / harsh11 / melody
    o_adv = dout("o_adv", (B, 3))          # softplus sums: local / phrase / global

    with tile.TileContext(nc) as tc:
        with (
            tc.tile_pool(name="const", bufs=1) as cp,
            tc.tile_pool(name="x", bufs=4) as xp,  # 16KB/part each
            tc.tile_pool(name="norm", bufs=4) as npo,
            tc.tile_pool(name="diag", bufs=4) as dgp,
            tc.tile_pool(name="junk", bufs=2) as jup,
            tc.tile_pool(name="small", bufs=1) as smp,
            tc.tile_pool(name="mus", bufs=1) as mup,
            tc.tile_pool(name="psum", bufs=1, space="PSUM") as psp,
        ):
            # ---- constants into SBUF
            msp_t = cp.tile([128, 4, 32], F32, tag="msp")
            msn_t = cp.tile([128, 4, 32], F32, tag="msn")
            pmp_t = cp.tile([128, PL], F32, tag="pmp")
            pmn_t = cp.tile([128, PL], F32, tag="pmn")
            onp_t = cp.tile([B, 1], F32, tag="onp")
            onn_t = cp.tile([B, 1], F32, tag="onn")
            nc.sync.dma_start(msp_t[:], msp.rearrange("g p m -> p g m"))
            nc.sync.dma_start(msn_t[:], msn.rearrange("g p m -> p g m"))
            nc.sync.dma_start(pmp_t[:], pmp[:])
            nc.sync.dma_start(pmn_t[:], pmn[:])
            nc.sync.dma_start(onp_t[:], onp[:])
            nc.sync.dma_start(onn_t[:], onn[:])

            # ---- output staging tiles
            PARTS = set(os.environ.get(
                "K_PARTS", "pairs,phrase,glob,mus,adv").split(","))
            ods_t = smp.tile([32, 20], F32, tag="ods")
            nc.vector.memset(ods_t[:], 0.0)
            omus_t = mup.tile([B, 4], F32, tag="omus")
            oadv_t = mup.tile([B, 3], F32, tag="oadv")
            nc.vector.memset(omus_t[:], 0.0)
            nc.vector.memset(oadv_t[:], 0.0)

            # ================= big pairs: local & input =================
            def sq_accum(k, xs, nsq_col, force_act=False):
                """nsq_col[128,1] = sum_d xs^2, on ACT or DVE."""
                xf = xs.bitcast(F32)
                if force_act or (k % 8) < ACT_OF_8:
                    junk = jup.tile([128, D], F32, tag="junk")
                    nc.scalar.activation(junk[:], xf, AF.Square,
                                         accum_out=nsq_col)
                else:
                    junk = jup.tile([128, D], F32, tag="junk")
                    nc.vector.scalar_tensor_tensor(
                        out=junk[:], in0=xf, scalar=1.0, in1=xf,
                        op0=ALU.mult, op1=ALU.mult, accum_out=nsq_col)

            def inv_norms(nsq, g):
                """inv[128,G] = sqrt(1/max(nsq, EPS2))"""
                G = nsq.shape()[1] if callable(getattr(nsq, "shape", None)) else None
                clip = npo.tile([128, g], F32, tag="clip")
                rec = npo.tile([128, g], F32, tag="rec")
                inv = npo.tile([128, g], F32, tag="inv")
                nc.vector.tensor_scalar(out=clip[:], in0=nsq, scalar1=EPS2,
                                        scalar2=None, op0=ALU.max)
                nc.vector.reciprocal(rec[:], clip[:])
                nc.scalar.activation(inv[:], rec[:], AF.Sqrt)
                return inv

            # tiles hold rows (16 b x 8 s); one 2 MiB contiguous DMA covers
            # 8 (bg, sg) group slices. DMAs alternate between the two HWDGE
            # rings (SP via nc.sync, ACT via nc.scalar) so the rings overlap.
            # matmul: psum[8g:8g+8, :] += sel[128,8].T @ x[128,512], where
            # sel = (+/-mask) * inv_norm; 4 accumulating MMs per s-group
            # slice (2 b-groups x fake/real).
            kglob = 0
            SGD = 8                         # (bg, sg) slices per DMA
            dma_engs = {"sync": [nc.sync],
                        "alt": [nc.sync, nc.scalar],
                        "alt3": [nc.sync, nc.scalar, nc.gpsimd]}[
                os.environ.get("K_DMA", "alt")]
            pair_list = [(fl, rl, (0, 1)), (fi, ri, (2, 3))]
            if "pairs" not in PARTS:
                pair_list = pair_list[:1] if "pair0" in PARTS else []
            for pair_i, (fake_src, real_src, cols) in enumerate(pair_list):
                # one [32, D] psum tile (own bank) per quad of 4 s-groups
                ps = [psp.tile([32, D], F32, tag=f"q{i}", name=f"psq{i}")
                      for i in range(8)]
                mm_cnt = [0] * 8
                for src, mask in ((fake_src, msp_t), (real_src, msn_t)):
                    for bg in range(2):          # b-groups of 16
                        for q in range(32 // SGD):         # 2 MiB chunks
                            xt = xp.tile([128, SGD * D], F32R, tag="x")
                            off = (bg * 32 + q * SGD) * D
                            dma_engs[(kglob // SGD) % len(dma_engs)].dma_start(
                                xt[:], src[:, off:off + SGD * D])
                            nsq = npo.tile([128, SGD], F32, tag="nsq")
                            tile_i = kglob // SGD
                            if (SQMODE == "seg"
                                    and tile_i % 8 >= ACT_TILES_OF_8):
                                xf = xt[:].bitcast(F32)
                                xsq = jup.tile([128, SGD, D], F32, tag="xsq")
                                nc.vector.tensor_mul(xsq[:], xf, xf)
                                nc.vector.tensor_reduce(
                                    nsq[:], xsq[:],
                                    axis=mybir.AxisListType.X, op=ALU.add)
                            else:
                                for k in range(SGD):
                                    sq_accum(kglob + k,
                                             xt[:, k * D:(k + 1) * D],
                                             nsq[:, k:k + 1],
                                             force_act=(SQMODE == "seg"))
                            kglob += SGD
                            inv = inv_norms(nsq[:], SGD)
                            for k in range(SGD):
                                gi = q * SGD + k          # global s-group
                                quad, gq = gi // 4, gi % 4
                                sel = dgp.tile([128, 32], F32R, tag="sel")
                                nc.gpsimd.tensor_scalar_mul(
                                    sel[:], mask[:, gq, :], inv[:, k:k + 1])
                                cnt = mm_cnt[quad]
                                nc.tensor.matmul(
                                    ps[quad][:], sel[:],
                                    xt[:, k * D:(k + 1) * D],
                                    start=(cnt == 0), stop=(cnt == 15))
                                mm_cnt[quad] = cnt + 1
                # drain the 8 quad banks -> per-s partial diff^2 sums
                for j in range(8):
                    col = pair_i * 8 + j
                    junk = jup.tile([32, D], F32, tag="junk32",
                                    name="junk32")
                    nc.scalar.activation(junk[:], ps[j][:], AF.Square,
                                         accum_out=ods_t[:, col:col + 1])

            # ================= phrase features =================
            phrase_list = ((fp, pmp_t), (rp, pmn_t)) if "phrase" in PARTS else ()
            if phrase_list:
                ps_p = psp.tile([PL, D], F32, tag="q0", name="ps_p")
            cnt = 0
            for src, mask in phrase_list:
                pt = xp.tile([128, 4 * D], F32R, tag="x")
                nc.sync.dma_start(pt[:], src[:])
                nsq = npo.tile([128, 4], F32, tag="nsq")
                for t in range(4):
                    junk = jup.tile([128, D], F32, tag="junk")
                    nc.scalar.activation(junk[:],
                                         pt[:, t * D:(t + 1) * D].bitcast(F32),
                                         AF.Square, accum_out=nsq[:, t:t + 1])
                inv = inv_norms(nsq[:], 4)
                for t in range(4):
                    selp = dgp.tile([128, PL], F32R, tag="selp")
                    nc.gpsimd.tensor_scalar_mul(selp[:], mask[:], inv[:, t:t + 1])
                    nc.tensor.matmul(
                        ps_p[:], selp[:], pt[:, t * D:(t + 1) * D],
                        start=(cnt == 0), stop=(cnt == 7))
                    cnt += 1
            if phrase_list:
                junk = jup.tile([128, D], F32, tag="junk")
                nc.scalar.activation(junk[:PL, :], ps_p[:], AF.Square,
                                     accum_out=ods_t[:PL, 16:17])

            # ================= global features =================
            glob_list = (((fg, onp_t), (rg, onn_t)) if "glob" in PARTS else ())
            if glob_list:
                ps_g = psp.tile([1, D], F32, tag="q1", name="ps_g")
            for gi, (src, ones) in enumerate(glob_list):
                gt = xp.tile([B, D], F32R, tag="gx")
                nc.sync.dma_start(gt[:], src[:])
                nsq = npo.tile([B, 1], F32, tag="gnsq")
                junk = jup.tile([128, D], F32, tag="junk")
                nc.scalar.activation(junk[:B, :], gt[:].bitcast(F32),
                                     AF.Square, accum_out=nsq[:])
                clip = npo.tile([B, 1], F32, tag="gclip")
                rec = npo.tile([B, 1], F32, tag="grec")
                inv = npo.tile([B, 1], F32, tag="ginv")
                nc.vector.tensor_scalar(out=clip[:], in0=nsq[:], scalar1=EPS2,
                                        scalar2=None, op0=ALU.max)
                nc.vector.reciprocal(rec[:], clip[:])
                nc.scalar.activation(inv[:], rec[:], AF.Sqrt)
                sel = npo.tile([B, 1], F32R, tag="gsel")
                nc.vector.tensor_scalar_mul(sel[:], ones[:], inv[:])
                nc.tensor.matmul(ps_g[:], sel[:], gt[:],
                                 start=(gi == 0), stop=(gi == 1))
            if glob_list:
                junk = jup.tile([128, D], F32, tag="junk")
                nc.scalar.activation(junk[:1, :], ps_g[:], AF.Square,
                                     accum_out=ods_t[:1, 17:18])

            # ================= musical perceptual =================
            if "mus" in PARTS:
                MUS_N = int(os.environ.get("K_MUS_N", "99"))
                tf = mup.tile([B, SL + 1], F32, tag="tf")
                nc.sync.dma_start(tf[:], tok[:])
                W = SL + 1

                def mtile(tag, w=W):
                    return mup.tile([B, w], F32, tag=tag, name=tag)

                a_t = mtile("m_a"); b_t = mtile("m_b"); ts_t = mtile("m_ts")
                nc.vector.tensor_scalar(out=a_t[:], in0=tf[:], scalar1=256.0,
                                        scalar2=None, op0=ALU.is_ge)
                nc.vector.tensor_scalar(out=b_t[:], in0=tf[:], scalar1=768.0,
                                        scalar2=None, op0=ALU.is_ge)
                nc.vector.tensor_sub(ts_t[:], a_t[:], b_t[:])
                junk_m = mtile("m_j1", SL)
                nc.vector.tensor_tensor(out=junk_m[:], in0=ts_t[:, 1:W],
                                        in1=ts_t[:, 0:SL], op=ALU.not_equal)
                nc.vector.tensor_reduce(omus_t[:, 0:1], junk_m[:],
                                        axis=mybir.AxisListType.X, op=ALU.add)

                c_t = mtile("m_c"); p_t = mtile("m_p"); pc_t = mtile("m_pc")
                nc.vector.tensor_scalar(out=c_t[:], in0=tf[:], scalar1=128.0,
                                        scalar2=None, op0=ALU.is_lt)
                nc.vector.tensor_mul(p_t[:], tf[:], c_t[:])
    # pc = p mod 12 via conditional subtraction (exact: p in [0,127])
                cur = p_t
                for ti, tval in enumerate((96.0, 48.0, 24.0, 12.0)):
                    ind_t = mtile(f"m_ind{ti}")
                    nc.vector.tensor_scalar(out=ind_t[:], in0=cur[:],
                                            scalar1=tval, scalar2=None,
                                            op0=ALU.is_ge)
                    nxt = pc_t if ti == 3 else mtile(f"m_mod{ti}")
                    nc.vector.scalar_tensor_tensor(
                        out=nxt[:], in0=ind_t[:], scalar=-tval, in1=cur[:],
                        op0=ALU.mult, op1=ALU.add)
                    cur = nxt

                # |d| == 6 or 11  <=>  d^2 == 36 or 121 (exact small ints)
                d_t = mtile("m_d", SL); iv_t = mtile("m_iv", SL)
                nc.vector.tensor_sub(d_t[:], pc_t[:, 0:SL], pc_t[:, 1:W])
                nc.vector.tensor_mul(iv_t[:], d_t[:], d_t[:])
                junk_m2 = mtile("m_j2", SL)
                nc.vector.tensor_scalar(out=junk_m2[:], in0=iv_t[:], scalar1=36.0,
                                        scalar2=None, op0=ALU.is_equal,
                                        op1=ALU.add, accum_out=omus_t[:, 1:2])
                junk_m3 = mtile("m_j3", SL)
                nc.vector.tensor_scalar(out=junk_m3[:], in0=iv_t[:], scalar1=121.0,
                                        scalar2=None, op0=ALU.is_equal,
                                        op1=ALU.add, accum_out=omus_t[:, 2:3])

                # |pdiff| > 12  <=>  pdiff^2 > 144 (exact: |pdiff| <= 127)
                pd_t = mtile("m_pd", SL); av_t = mtile("m_av", SL)
                nc.vector.tensor_sub(pd_t[:], p_t[:, 0:SL], p_t[:, 1:W])
                nc.vector.tensor_mul(av_t[:], pd_t[:], pd_t[:])
                junk_m4 = mtile("m_j4", SL)
                nc.vector.tensor_scalar(out=junk_m4[:], in0=av_t[:], scalar1=144.0,
                                        scalar2=None, op0=ALU.is_gt,
                                        op1=ALU.add, accum_out=omus_t[:, 3:4])

            # ================= adversarial =================
            if "adv" in PARTS:
                ll_t = mup.tile([B, SL], F32, tag="ll")
                pl_t = mup.tile([B, PL], F32, tag="pl")
                gl_t = mup.tile([B, 1], F32, tag="gl")
                nc.sync.dma_start(ll_t[:], ll[:])
                nc.sync.dma_start(pl_t[:], pl[:])
                nc.sync.dma_start(gl_t[:], gl[:])
    # softplus(-x) = ln(1 + exp(-x)); Exp and Ln share one ACT table set
                for col, (src_t, w) in enumerate(((ll_t, SL), (pl_t, PL),
                                                  (gl_t, 1))):
                    e_t = mup.tile([B, w], F32, tag=f"a_e{col}", name=f"a_e{col}")
                    nc.scalar.activation(e_t[:], src_t[:], AF.Exp, scale=-1.0)
                    junk_a = mup.tile([B, w], F32, tag=f"a_j{col}",
                                      name=f"a_j{col}")
                    nc.scalar.activation(junk_a[:], e_t[:], AF.Ln, bias=1.0,
                                         accum_out=oadv_t[:, col:col + 1])

            # ================= outputs =================
            nc.sync.dma_start(o_ds[:], ods_t[:])
            nc.sync.dma_start(o_mus[:], omus_t[:])
            nc.sync.dma_start(o_adv[:], oadv_t[:])

    in_names = ["rl", "fl", "ri", "fi", "rp", "fp", "rg", "fg",
                "ll", "pl", "gl", "tok", "idp", "idn", "pmp", "pmn",
                "onp", "onn"]
    out_names = ["o_ds", "o_mus", "o_adv"]
    return in_names, out_names


_CACHE = {}


def _get_nc():
    if "nc" not in _CACHE:
        nc = bacc.Bacc("TRN2", target_bir_lowering=False, debug=False,
                       enable_asserts=False, num_devices=NCORES)
        _build(nc)
        nc.compile()
        _CACHE["nc"] = nc
    return _CACHE["nc"]


def _shard_inputs(inputs):
    """Build the 8 per-core input maps from the full input dict."""
    f32 = np.float32
    C = np.ascontiguousarray

    msp = np.zeros((4, 128, 32), dtype=f32)
    for g in range(4):
        for r in range(128):
            msp[g, r, 8 * g + r % 8] = 1.0
    msn = -msp
    pmp = np.zeros((128, PL), dtype=f32)
    for r in range(128):
        pmp[r, r % PL] = 1.0
    pmn = -pmp
    onp = np.ones((B, 1), dtype=f32)
    onn = -onp

    tokens = inputs["tokens"]
    tok_f = tokens.astype(f32)

    in_maps = []
    for c in range(NCORES):
        s0, s1 = c * SL, (c + 1) * SL
        p0, p1 = c * PL, (c + 1) * PL
        if s1 < S:
            tok_c = tok_f[:, s0:s1 + 1]
        else:
            tok_c = np.concatenate([tok_f[:, s0:s1], tok_f[:, s1 - 1:s1]],
                                   axis=1)
        def bigT(x):
            # (B, SL, D) -> [128, (bg, sg, d)] contiguous:
            #   row (b16, s8), col ((bg*32 + sg)*D + d)
            y = x[:, s0:s1, :].reshape(2, 16, 32, 8, x.shape[-1])
            return C(y.transpose(1, 3, 0, 2, 4).reshape(128, 2 * 32 * D))

        def phrT(x):
            # (B, PL, D) -> [128, (t, d)]: row (e8, p16), col (t*D + d)
            # where b = t*8 + e
            y = x[:, p0:p1, :].reshape(4, 8, PL, D)
            return C(y.transpose(1, 2, 0, 3).reshape(128, 4 * D))

        in_maps.append({
            "rl": bigT(inputs["real_local"]),
            "fl": bigT(inputs["fake_local"]),
            "ri": bigT(inputs["real_input"]),
            "fi": bigT(inputs["fake_input"]),
            "rp": phrT(inputs["real_phrase"]),
            "fp": phrT(inputs["fake_phrase"]),
            "rg": C(inputs["real_global"]),
            "fg": C(inputs["fake_global"]),
            "ll": C(inputs["local_logits"][:, s0:s1]),
            "pl": C(inputs["phrase_logits"][:, p0:p1]),
            "gl": C(inputs["global_logits"]),
            "tok": C(tok_c),
            "msp": msp, "msn": msn, "pmp": pmp, "pmn": pmn,
            "onp": onp, "onn": onn,
        })
    return in_maps


def _combine(results):
    """Combine per-core partial sums (float64) into the final scalar."""
    ds = [r["o_ds"].astype(np.float64) for r in results]
    mus = [r["o_mus"].astype(np.float64) for r in results]
    adv = [r["o_adv"].astype(np.float64) for r in results]

    ss_l = sum(d[:, 0:8].sum() for d in ds)
    ss_i = sum(d[:, 8:16].sum() for d in ds)
    ss_p = sum(d[:, 16].sum() for d in ds)
    ss_g = ds[0][:, 17].sum()

    m_l = ss_l / (B * B * S * D)
    m_i = ss_i / (B * B * S * D)
    m_p = ss_p / (B * B * P * D)
    m_g = ss_g / (B * B * D)
    fm = (0.4 * m_l + 0.4 * m_p + 0.2 * m_g + 0.1 * m_i) / 4.0

    rhythm = sum(m[:, 0].sum() for m in mus) / (B * (S - 1))
    harmony = sum(m[:, 1].sum() + m[:, 2].sum() for m in mus) / (B * S)
    melody = sum(m[:, 3].sum() for m in mus) / (B * (S - 1))
    musical = rhythm + harmony + melody

    a_l = sum(a[:, 0].sum() for a in adv) / (B * S)
    a_p = sum(a[:, 1].sum() for a in adv) / (B * P)
    a_g = adv[0][:, 2].sum() / B
    advt = 0.4 * a_l + 0.4 * a_p + 0.2 * a_g

    return np.float32(fm + musical + advt)


def _numpy_core(im):
    """Pure-numpy model of one core's device program (for debugging)."""
    out = {}
    ds = np.zeros((32, 20), np.float64)

    def normed_diff(fake, real, nrows):
        # rows laid out as in the device program
        f = fake.reshape(-1, D).astype(np.float64)
        r = real.reshape(-1, D).astype(np.float64)
        fi = 1.0 / np.maximum(np.sqrt((f * f).sum(-1)), 1e-12)
        ri = 1.0 / np.maximum(np.sqrt((r * r).sum(-1)), 1e-12)
        return f * fi[:, None], r * ri[:, None]

    for pair, (fk, rk) in ((0, ("fl", "rl")), (1, ("fi", "ri"))):
        # im[fk] is [128, (bg, sg, d)]; reassemble to (B, SL, D)
        def unT(y):
            z = y.reshape(16, 8, 2, 32, D)          # b, s, bg, sg, d
            return z.transpose(2, 0, 3, 1, 4).reshape(B, SL, D)
        f = unT(im[fk]).astype(np.float64)
        r = unT(im[rk]).astype(np.float64)
        fn = f / np.maximum(np.linalg.norm(f, axis=-1, keepdims=True), 1e-12)
        rn = r / np.maximum(np.linalg.norm(r, axis=-1, keepdims=True), 1e-12)
        acc = fn.sum(0) - rn.sum(0)          # [SL, D]
        for j in range(8):
            blk = acc[j * 32:(j + 1) * 32]   # [32, D]
            ds[:, pair * 8 + j] += (blk * blk).sum(-1)
    def unP(y):
        z = y.reshape(8, PL, 4, D)               # e, p, t, d
        return z.transpose(2, 0, 1, 3).reshape(B, PL, D)
    f = unP(im["fp"]).astype(np.float64)
    r = unP(im["rp"]).astype(np.float64)
    fn = f / np.maximum(np.linalg.norm(f, axis=-1, keepdims=True), 1e-12)
    rn = r / np.maximum(np.linalg.norm(r, axis=-1, keepdims=True), 1e-12)
    acc = fn.sum(0) - rn.sum(0)              # [PL, D]
    ds[:PL, 16] = (acc * acc).sum(-1)
    f, r = im["fg"].astype(np.float64), im["rg"].astype(np.float64)
    fn = f / np.maximum(np.linalg.norm(f, axis=-1, keepdims=True), 1e-12)
    rn = r / np.maximum(np.linalg.norm(r, axis=-1, keepdims=True), 1e-12)
    acc = fn.sum(0) - rn.sum(0)              # [D]
    ds[0, 17] = (acc * acc).sum()
    out["o_ds"] = ds

    t = im["tok"].astype(np.float64)
    tsh = ((t >= 256) & (t < 768)).astype(np.float64)
    mus = np.zeros((B, 4), np.float64)
    mus[:, 0] = np.abs(np.diff(tsh, axis=1)).sum(1)
    p = t * (t < 128)
    pc = np.mod(p, 12.0)
    iv = np.abs(pc[:, :-1] - pc[:, 1:])
    mus[:, 1] = (iv == 6).sum(1)
    mus[:, 2] = (iv == 11).sum(1)
    pd = np.abs(p[:, :-1] - p[:, 1:])
    mus[:, 3] = (pd > 12).sum(1)
    out["o_mus"] = mus

    sp = lambda x: np.log1p(np.exp(-np.abs(x))) + np.maximum(-x, 0.0)
    adv = np.zeros((B, 3), np.float64)
    adv[:, 0] = sp(im["ll"].astype(np.float64)).sum(1)
    adv[:, 1] = sp(im["pl"].astype(np.float64)).sum(1)
    adv[:, 2] = sp(im["gl"].astype(np.float64)).sum(1)
    out["o_adv"] = adv
    return out


def _run(inputs, backend="hw", trace=False):
    """Returns (scalar_result, exec_time_ns_or_None, raw_results)."""
    in_maps = _shard_inputs(inputs)
    if backend == "numpy":
        results = [_numpy_core(im) for im in in_maps]
        return _combine(results), None, results
    nc = _get_nc()
    if backend == "sim":
        from concourse.bass_interp import CoreSim
        results = []
        for im in in_maps:
            sim = CoreSim(nc, trace=False)
            for k, v in im.items():
                sim.tensor(k)[:] = v
            sim.simulate()
            results.append({k: np.array(sim.tensor(k))
                            for k in ("o_ds", "o_mus", "o_adv")})
        return _combine(results), None, results
    from concourse.bass_utils import run_bass_kernel_spmd
    br = run_bass_kernel_spmd(nc, in_maps, list(range(NCORES)), trace=trace)
    return _combine(br.results), br.exec_time_ns, br.results


def kernel(**inputs) -> np.ndarray:
    result, _, _ = _run(inputs, backend="hw")
    return result

